# revision 39
# baseline (speedup 1.0000x reference)
"""DGL capsule routing layer on 8 trn2 NeuronCores (Bass/Tile).

Math: for routing_num iterations,
    c = softmax(b, axis=out)                        # b0 = 0
    s = einsum('io,iof->of', c, uh)
    v = squash(s)
    b = b + einsum('iof,of->io', uh, v)
Output: final v [OUT, F].

Identity: b_t = uh . (v_1 + ... + v_{t-1}) = uh . w, so b is recomputed
from the cumulative w each pass instead of being materialized in DRAM.

Sharding: OUT_NODES split across 8 cores (128 o's per core); every core
holds ALL 4096 in-nodes of its o-slice. Softmax over o needs a cross-core
sum of the per-i denominators only: AllReduce of [128,16] f32 (8 KiB) per
half-pass, overlapped with the other half's compute. s/v/squash are then
fully core-local; the final output is a host-side concat of the v-shards.

Memory plan: the per-core shard (4096 x 128 x 16 f32 = 32 MiB) is read
from HBM once (pass 1) and kept resident in SBUF as bf16 (16 MiB) in
f-major layout [i-part, (f,o)], held as 32 per-i-block tiles so the tile
dependency tracker doesn't serialize unrelated readers/writers:
  - f-major makes every hot DVE op contiguous step-1 bf16 => 2x DVE mode
    (tensor_reduce only runs at 1x, so the f-sum is a 4-level TT tree).
  - per i-block (128 i's): tm = uh*w_bcast (TT); tree over f-halves -> b
    [128,128] f32; ACT exp with accum_out -> e (bf16) + den column.
  - p = e (bcast over f) * uh (TT 2x); PE matmul with rinv[i] as the
    1-column bf16 stationary accumulates s [1,2048] over all 32 blocks
    in a single PSUM bank (one flush per pass).
  - squash runs partition-spread [128o,16f] via a tiny DRAM round trip;
    sqrt is a DVE Babylonian iteration (no Ln table -> the Exp ACT table
    loads exactly once); w broadcasts back via DMA-broadcast + cast.
"""

import numpy as np
from contextlib import ExitStack

import concourse.bass as bass
import concourse.mybir as mybir
import concourse.tile as tile
from concourse import bacc
from concourse import bass_utils

F32 = mybir.dt.float32
BF16 = mybir.dt.bfloat16
AX = mybir.AxisListType
AF = mybir.ActivationFunctionType

IN_NODES, OUT_NODES, F_SIZE = 4096, 1024, 16
CORES = 8
O_LOC = OUT_NODES // CORES         # 128 out-nodes per core
ROWL = O_LOC * F_SIZE              # 2048 elems per in-node row (local)
P = 128
NBLK = IN_NODES // P               # 32 i-blocks per core
NMM = ROWL // 512                  # 4 matmuls per block (psum bank = 512 f32)
# AR chunks sized so every chunk's AllReduce (~25-30us incl floor) lands
# before the p-phase consumer reaches it
SPLITS = [(0, 12), (12, 24), (24, 32)]
SQRT_ITERS = 5


def _body(nc, tc, uh, v_out, R, rg):
    uh_t = uh.rearrange("(n p) r -> n p r", p=P)   # [32, 128, 2048]

    with ExitStack() as ctx:
        stage = ctx.enter_context(tc.tile_pool(name="stage", bufs=2))
        work = ctx.enter_context(tc.tile_pool(name="work", bufs=2))
        ppool = ctx.enter_context(tc.tile_pool(name="ppool", bufs=2))
        small = ctx.enter_context(tc.tile_pool(name="small", bufs=2))
        sflush = ctx.enter_context(tc.tile_pool(name="sflush", bufs=1))
        persist = ctx.enter_context(tc.tile_pool(name="persist", bufs=1))
        pspool = ctx.enter_context(tc.tile_pool(name="pspool", bufs=1, space="PSUM"))
        dram = ctx.enter_context(tc.tile_pool(name="dram", bufs=2, space="DRAM"))

        # resident shard in 4 quarter-tiles: pass-1 matmul bursts over
        # quarter N overlap the cast stream filling quarter N+1
        QBLK = NBLK // 4
        res_q = [persist.tile([P, QBLK * ROWL], BF16, name=f"res{q}")
                 for q in range(4)]

        def res_sl(blk):
            return res_q[blk // QBLK][
                :, (blk % QBLK) * ROWL:(blk % QBLK + 1) * ROWL]

        e_all = persist.tile([P, NBLK * P], BF16, name="e_all")
        den_h = [persist.tile([P, b1 - b0], F32, name=f"den{h}")
                 for h, (b0, b1) in enumerate(SPLITS)]
        rinv_h = [persist.tile([P, b1 - b0], BF16, name=f"rinv{h}")
                  for h, (b0, b1) in enumerate(SPLITS)]
        w_bcast = persist.tile([P, ROWL], BF16, name="w_bcast")
        w_flat = persist.tile([1, ROWL], BF16, name="w_flat")

        c0f = persist.tile([P, 1], F32, name="c0f")
        nc.vector.memset(c0f, 1.0 / OUT_NODES)
        c0 = persist.tile([P, 1], BF16, name="c0")
        nc.vector.tensor_copy(c0, c0f)
        onesf = persist.tile([1, P], F32, name="onesf")
        nc.vector.memset(onesf, 1.0)
        ones_bf = persist.tile([1, P], BF16, name="ones_bf")
        nc.vector.tensor_copy(ones_bf, onesf)

        def chunk_of(blk):
            for h, (a, b) in enumerate(SPLITS):
                if a <= blk < b:
                    return h, blk - a

        for t in range(1, R + 1):
            s_ps = pspool.tile([1, ROWL], F32, tag="s_ps")
            if t == 1:
                # stream from HBM, cast+shuffle (o,f)->(f,o) into residency,
                # and run the uniform-coupling s-matmuls off the fresh tiles
                for q in range(4):
                    for blk in range(q * QBLK, (q + 1) * QBLK):
                        st = stage.tile([P, ROWL], F32, tag="st")
                        nc.sync.dma_start(st, uh_t[blk])
                        nc.vector.tensor_copy(
                            res_sl(blk).rearrange("p (f o) -> p f o",
                                                  o=O_LOC),
                            st.rearrange("p (o f) -> p f o", f=F_SIZE),
                        )
                    # dense matmul burst over the finished quarter
                    for blk in range(q * QBLK, (q + 1) * QBLK):
                        rs = res_sl(blk)
                        for c in range(NMM):
                            nc.tensor.matmul(
                                s_ps[:, c * 512:(c + 1) * 512],
                                c0,
                                rs[:, c * 512:(c + 1) * 512],
                                start=(blk == 0), stop=(blk == NBLK - 1),
                                skip_group_check=True,
                            )
            else:
                # b-phase: b = sum_f uh*w, e = exp(b), den-accum; AR per half
                for h, (b0, b1) in enumerate(SPLITS):
                    for j in range(b1 - b0):
                        blk = b0 + j
                        rs = res_sl(blk)
                        tm = work.tile([P, ROWL], BF16, tag="tm")
                        nc.vector.tensor_mul(tm, rs, w_bcast)
                        l1 = work.tile([P, 1024], BF16, tag="l1")
                        nc.vector.tensor_add(l1, tm[:, :1024], tm[:, 1024:])
                        l2 = work.tile([P, 512], BF16, tag="l2")
                        nc.vector.tensor_add(l2, l1[:, :512], l1[:, 512:])
                        l3 = work.tile([P, 256], BF16, tag="l3")
                        nc.vector.tensor_add(l3, l2[:, :256], l2[:, 256:])
                        bb = work.tile([P, P], F32, tag="bb")
                        nc.vector.tensor_add(bb, l3[:, :128], l3[:, 128:])
                        nc.scalar.activation(
                            e_all[:, blk * P:(blk + 1) * P], bb, AF.Exp,
                            accum_out=den_h[h][:, j:j + 1])
                    nch = b1 - b0
                    ar_in = dram.tile([P, nch], F32, tag=f"ar_in{h}")
                    nc.sync.dma_start(ar_in, den_h[h])
                    ar_out = dram.tile([P, nch], F32, tag=f"ar_out{h}")
                    nc.gpsimd.collective_compute(
                        "AllReduce", mybir.AluOpType.add, replica_groups=rg,
                        ins=[ar_in.opt()], outs=[ar_out.opt()],
                    )
                    deng = small.tile([P, nch], F32, tag="deng")
                    nc.sync.dma_start(deng, ar_out)
                    rf = small.tile([P, nch], F32, tag="rf")
                    nc.vector.reciprocal(rf, deng)
                    nc.vector.tensor_copy(rinv_h[h], rf)
                # p-phase: p = e (bcast over f) * uh; s += rinv^T @ p
                for blk in range(NBLK):
                    h, j = chunk_of(blk)
                    rs = res_sl(blk)
                    p = ppool.tile([P, ROWL], BF16, tag="p")
                    e_sl = e_all[:, blk * P:(blk + 1) * P]
                    nc.vector.tensor_mul(
                        p.rearrange("p (f o) -> p f o", o=O_LOC),
                        rs.rearrange("p (f o) -> p f o", o=O_LOC),
                        e_sl[:, None, :].broadcast_to([P, F_SIZE, O_LOC]),
                    )
                    for c in range(NMM):
                        nc.tensor.matmul(
                            s_ps[:, c * 512:(c + 1) * 512],
                            rinv_h[h][:, j:j + 1],
                            p[:, c * 512:(c + 1) * 512],
                            start=(blk == 0), stop=(blk == NBLK - 1),
                            skip_group_check=True,
                        )

            # tail: merge the two psum rows, squash flat on partition 0
            # (no DRAM round trips), broadcast w via a k=1 ones-matmul
            s_sb = sflush.tile([1, ROWL], F32, tag="s_sb")
            nc.scalar.copy(s_sb, s_ps)
            # sq[o] = sum_f s^2 via in-place halving tree over f-slabs
            ssq = sflush.tile([1, ROWL], F32, tag="ssq")
            nc.vector.tensor_mul(ssq, s_sb, s_sb)
            nc.vector.tensor_add(ssq[:, :1024], ssq[:, :1024], ssq[:, 1024:])
            nc.vector.tensor_add(ssq[:, :512], ssq[:, :512], ssq[:, 512:1024])
            nc.vector.tensor_add(ssq[:, :256], ssq[:, :256], ssq[:, 256:512])
            nc.vector.tensor_add(ssq[:, :128], ssq[:, :128], ssq[:, 128:256])
            sq = ssq[:, :O_LOC]
            # sqrt(sq) via Babylonian iteration y <- (y + sq/y)/2 from
            # y0 = (1+sq)/2 >= sqrt(sq); DVE-only (keeps ACT on one table)
            d1 = sflush.tile([1, O_LOC], F32, tag="d1")
            nc.vector.tensor_scalar_add(d1, sq, 1.0)
            y = sflush.tile([1, O_LOC], F32, tag="y")
            nc.vector.tensor_scalar_mul(y, d1, 0.5)
            for _ in range(SQRT_ITERS):
                ry = sflush.tile([1, O_LOC], F32, tag="ry")
                nc.vector.reciprocal(ry, y)
                q = sflush.tile([1, O_LOC], F32, tag="q")
                nc.vector.tensor_mul(q, sq, ry)
                nc.vector.tensor_add(q, q, y)
                nc.vector.tensor_scalar_mul(y, q, 0.5)
            rd = sflush.tile([1, O_LOC], F32, tag="rd")
            nc.vector.reciprocal(rd, d1)
            sc = sflush.tile([1, O_LOC], F32, tag="sc")
            nc.vector.tensor_mul(sc, y, rd)
            # v = s * sc (sc broadcast over f), in place over s_sb
            nc.vector.tensor_mul(
                s_sb.rearrange("a (f o) -> a f o", o=O_LOC),
                s_sb.rearrange("a (f o) -> a f o", o=O_LOC),
                sc[:, None, :].broadcast_to([1, F_SIZE, O_LOC]),
            )
            if t == R:
                v_dram = dram.tile([ROWL], F32, tag="v_dram")
                nc.sync.dma_start(v_dram.unsqueeze(0), s_sb)
                v2 = small.tile([P, F_SIZE], F32, tag="v2")
                nc.sync.dma_start(
                    v2, v_dram.rearrange("(f o) -> o f", o=O_LOC))
                nc.sync.dma_start(v_out, v2)
            else:
                if t == 1:
                    nc.vector.tensor_copy(w_flat, s_sb)
                else:
                    nc.vector.tensor_add(w_flat, w_flat, s_sb)
                ps_w = pspool.tile([P, ROWL], F32, tag="ps_w")
                for c in range(NMM):
                    nc.tensor.matmul(
                        ps_w[:, c * 512:(c + 1) * 512],
                        ones_bf,
                        w_flat[:, c * 512:(c + 1) * 512],
                        start=True, stop=True,
                        skip_group_check=True,
                    )
                nc.vector.tensor_copy(w_bcast, ps_w)


def _build(routing_num: int):
    R = int(routing_num)
    assert R >= 1
    nc = bacc.Bacc(
        "TRN2", target_bir_lowering=False, debug=False, num_devices=CORES)
    uh = nc.dram_tensor("uh", [IN_NODES, ROWL], F32, kind="ExternalInput")
    v_out = nc.dram_tensor("v_out", [O_LOC, F_SIZE], F32,
                           kind="ExternalOutput")
    rg = [list(range(CORES))]
    with tile.TileContext(nc) as tc:
        _body(nc, tc, uh.ap(), v_out.ap(), R, rg)
    nc.compile()
    return nc


_CACHE: dict = {}


def _get_nc(routing_num: int):
    R = int(routing_num)
    if R not in _CACHE:
        _CACHE[R] = _build(R)
    return _CACHE[R]


def _shard(u_hat: np.ndarray):
    uh = np.asarray(u_hat, dtype=np.float32)
    assert uh.shape == (IN_NODES * OUT_NODES, F_SIZE), uh.shape
    uh3 = uh.reshape(IN_NODES, OUT_NODES, F_SIZE)
    return [
        {"uh": np.ascontiguousarray(
            uh3[:, k * O_LOC:(k + 1) * O_LOC, :]).reshape(IN_NODES, ROWL)}
        for k in range(CORES)
    ]


def run(u_hat, routing_num, trace=False):
    nc = _get_nc(routing_num)
    in_maps = _shard(u_hat)
    res = bass_utils.run_bass_kernel_spmd(
        nc, in_maps, core_ids=list(range(CORES)), trace=trace)
    return res


def gather(res) -> np.ndarray:
    return np.concatenate(
        [np.asarray(res.results[c]["v_out"], dtype=np.float32)
         for c in range(CORES)], axis=0)


def kernel(u_hat, routing_num):
    res = run(u_hat, routing_num, trace=False)
    return gather(res)
